# revision 1
# baseline (speedup 1.0000x reference)
"""Trainium2 Bass kernel for CAM-style channel attention module.

Reference computation (per batch b):
    Q  = W @ X + bias          # 1x1 conv: [256,512]@[512,4096] -> [256,4096]
    E  = Q @ X^T / sqrt(4096)  # [256,512] channel-attention energy
    A  = softmax(E, axis=-1)
    out = gamma * (A @ X) + Q  # residual

Key algebraic trick: the residual never needs Q materialized ——
    gamma*(A@X) + (W@X + b) = (W + gamma*A) @ X + b
so the final stage is a single fused matmul with combined weights.

Device strategy (8 NeuronCores, data-parallel over batch, 2 batches/core).
All matmuls bf16 with fp32 PSUM accumulation. Per batch:
  1. DMA-cast X fp32->bf16 (SWDGE inline cast) in progressive column chunks
     (4x256 then 6x512) so PE can start after the first chunk lands.
  2. Fused main loop over 32 n-tiles: the SAME stationary X-tile feeds
     (a) a transpose as a REGULAR matmul against identity (keeps the PE HAM
         clock-gate warm, unlike transpose-mode) building XT
     (b) the QT matmul (rhs=W^T) building QT = X^T W^T (+b on evacuation)
     and the energy matmuls E[q,:] += QT[n-tile,q]^T XT[n-tile,:] are
     interleaved with a 1-iteration lag so PE never stalls on evacuations.
  3. softmax on ScalarE (exp with fused accum row-sums) / VectorE;
     gamma and 1/rowsum fold into one per-row scale of A.
  4. A^T via regular matmul vs identity; lhsT_final = W^T + scaled-A^T.
  5. final = lhsT_final^T @ X (+b via bias-add on PSUM evacuation,
     alternating ScalarE/VectorE).
"""

import numpy as np
import ml_dtypes

import concourse.bass as bass
import concourse.tile as tile
from concourse import bacc, mybir
from concourse.bass_utils import run_bass_kernel_spmd

P = 128
NB = 2        # batches per core (B=16 over 8 cores)
C = 512       # input channels
C1 = 256      # conv output channels
HW = 4096     # H*W
CT = C // P   # 4 c-tiles
NT = HW // P  # 32 n-tiles
QT = C1 // P  # 2 q-tiles
NCHUNK = 512  # matmul free-dim chunk (one PSUM bank)
# x DMA column chunk widths: small leading chunks so PE starts early
XCHUNKS = [256] * 4 + [512] * 6
XBOUND = [0]
for _w in XCHUNKS:
    XBOUND.append(XBOUND[-1] + _w)
assert XBOUND[-1] == HW
F32 = mybir.dt.float32
BF16 = mybir.dt.bfloat16
SCALE = 1.0 / 64.0  # 1/sqrt(HW)

N_CORES = 8


def build_nc():
    nc = bacc.Bacc("TRN2", target_bir_lowering=False, debug=False,
                   num_devices=N_CORES)

    xs = nc.dram_tensor("xs", [NB, C, HW], F32, kind="ExternalInput").ap()
    wt_f = nc.dram_tensor("wt_f", [P, CT, C1], F32, kind="ExternalInput").ap()
    wt_b = nc.dram_tensor("wt_b", [P, CT, C1], BF16, kind="ExternalInput").ap()
    bbc = nc.dram_tensor("bbc", [P, C1], F32, kind="ExternalInput").ap()
    bq = nc.dram_tensor("bq", [P, QT], F32, kind="ExternalInput").ap()
    gam = nc.dram_tensor("gam", [P, 1], F32, kind="ExternalInput").ap()
    out = nc.dram_tensor("out", [NB, C1, HW], F32, kind="ExternalOutput").ap()

    ident_dram = nc.inline_tensor(np.eye(P, dtype=ml_dtypes.bfloat16),
                                  name="ident")

    with tile.TileContext(nc) as tc:
        with (
            tc.tile_pool(name="const", bufs=1) as const,
            tc.tile_pool(name="xb", bufs=2 * len(XCHUNKS)) as xb_pool,
            tc.tile_pool(name="xtq", bufs=8) as xtq_pool,
            tc.tile_pool(name="sm", bufs=2) as sm_pool,
            tc.tile_pool(name="lhsf", bufs=2) as lhsf_pool,
            tc.tile_pool(name="osb", bufs=3) as osb_pool,
            tc.tile_pool(name="psA", bufs=2, space="PSUM") as psA,
            tc.tile_pool(name="psB", bufs=2, space="PSUM") as psB,
            tc.tile_pool(name="psC", bufs=2, space="PSUM") as psC,
        ):
            # ---- constants (all plain HWDGE loads; host pre-broadcast) ----
            ident = const.tile([P, P], BF16)
            nc.sync.dma_start(out=ident, in_=ident_dram.ap())
            wtb_sb = const.tile([P, CT, C1], BF16)
            nc.sync.dma_start(out=wtb_sb, in_=wt_b)
            wtf_sb = const.tile([P, CT, C1], F32)
            nc.sync.dma_start(out=wtf_sb, in_=wt_f)
            bbc_sb = const.tile([P, C1], F32)
            nc.sync.dma_start(out=bbc_sb, in_=bbc)
            bq_sb = const.tile([P, QT], F32)
            nc.sync.dma_start(out=bq_sb, in_=bq)
            gam_sb = const.tile([P, 1], F32)
            nc.sync.dma_start(out=gam_sb, in_=gam)

            x_r = xs.rearrange("b (ct p) n -> b p ct n", p=P)
            out_r = out.rearrange("b (t p) n -> b p t n", p=P)

            # ================= software-pipelined batch schedule ==========
            # PE order: BC(0) | BC(1)[0:16] AT0 F0[0:2] | BC(1)[16:32]
            #           F0[2:4] AT1 F1 — softmax latencies hide under the
            #           other batch's matmul streams; PE never idles.
            st = [dict() for _ in range(NB)]

            def issue_x_dma(bi):
                xb_ch = []
                for j, w in enumerate(XCHUNKS):
                    cht = xb_pool.tile([P, CT, max(XCHUNKS)], BF16, tag="xb",
                                       name=f"xb_{bi}_{j}")
                    nc.gpsimd.dma_start(
                        out=cht[:, :, :w],
                        in_=x_r[bi][:, :, XBOUND[j]:XBOUND[j + 1]])
                    xb_ch.append(cht)
                st[bi]["xb"] = xb_ch

            def xb_slices(bi, ct, lo, width):
                """Slices covering [lo, lo+width) split at chunk bounds.
                Yields (col_offset_in_request, sbuf_slice)."""
                end = lo + width
                out_sl = []
                for j, w in enumerate(XCHUNKS):
                    clo, chi = XBOUND[j], XBOUND[j + 1]
                    if chi <= lo or clo >= end:
                        continue
                    a, b = max(lo, clo), min(end, chi)
                    out_sl.append(
                        (a - lo,
                         st[bi]["xb"][j][:, ct, a - clo:b - clo]))
                return out_sl

            def xb_slice(bi, ct, lo, width):
                sl = xb_slices(bi, ct, lo, width)
                assert len(sl) == 1
                return sl[0][1]

            def emit_B(bi, nt):
                ps_xt = psA.tile([P, C], F32, tag="xt")
                ps_qt = psA.tile([P, C1], F32, tag="qt")
                for ct in range(CT):
                    xtile = xb_slice(bi, ct, nt * P, P)
                    nc.tensor.matmul(ps_xt[:, ct * P:(ct + 1) * P],
                                     xtile, ident, start=True, stop=True)
                    nc.tensor.matmul(ps_qt, xtile, wtb_sb[:, ct, :],
                                     start=(ct == 0), stop=(ct == CT - 1))
                xt_t = xtq_pool.tile([P, C], BF16, tag="xt_sb")
                qt_t = xtq_pool.tile([P, C1], BF16, tag="qt_sb")
                nc.scalar.copy(out=xt_t, in_=ps_xt)
                nc.vector.tensor_add(out=qt_t, in0=ps_qt, in1=bbc_sb)
                return xt_t, qt_t

            def emit_C(bi, nt, xt_t, qt_t):
                for qi in range(QT):
                    nc.tensor.matmul(
                        st[bi]["ps_e"][qi], qt_t[:, qi * P:(qi + 1) * P],
                        xt_t, start=(nt == 0), stop=(nt == NT - 1))

            LAG = 3  # energy matmuls trail the B stage by LAG n-tiles

            def emit_BC_range(bi, lo, hi):
                # fused B + energy with a LAG-iteration lag so the PE never
                # waits on the ScalarE/VectorE PSUM evacuations
                if lo == 0:
                    st[bi]["ps_e"] = [
                        psB.tile([P, C], F32, tag="e", name=f"ps_e{bi}{qi}")
                        for qi in range(QT)]
                    st[bi]["pend"] = []
                for nt in range(lo, hi):
                    cur = emit_B(bi, nt)
                    st[bi]["pend"].append((nt, cur))
                    if len(st[bi]["pend"]) > LAG:
                        pnt, ptiles = st[bi]["pend"].pop(0)
                        emit_C(bi, pnt, *ptiles)
                if hi == NT:
                    for pnt, ptiles in st[bi]["pend"]:
                        emit_C(bi, pnt, *ptiles)
                    st[bi]["pend"] = []

            def emit_softmax(bi):
                a_scaled = sm_pool.tile([P, QT, C], BF16, tag="a",
                                        name=f"a_scaled{bi}")
                for qi in range(QT):
                    ps_e = st[bi]["ps_e"][qi]
                    mx = sm_pool.tile([P, 1], F32, tag="mx")
                    nc.vector.reduce_max(mx, ps_e,
                                         axis=mybir.AxisListType.X,
                                         negate=True)
                    nbias = sm_pool.tile([P, 1], F32, tag="nb")
                    nc.vector.tensor_scalar_mul(nbias, mx, SCALE)
                    a_f = sm_pool.tile([P, C], F32, tag="af")
                    rs = sm_pool.tile([P, 1], F32, tag="rs")
                    nc.scalar.activation(
                        out=a_f, in_=ps_e,
                        func=mybir.ActivationFunctionType.Exp,
                        bias=nbias, scale=SCALE, accum_out=rs)
                    rc = sm_pool.tile([P, 1], F32, tag="rc")
                    nc.vector.reciprocal(rc, rs)
                    sc = sm_pool.tile([P, 1], F32, tag="sc")
                    nc.vector.tensor_mul(sc, rc, gam_sb)
                    nc.vector.tensor_scalar_mul(a_scaled[:, qi, :], a_f, sc)
                st[bi]["a"] = a_scaled

            def emit_ATcombine(bi):
                lhsf = lhsf_pool.tile([P, CT, C1], BF16, name=f"lhsf{bi}")
                a_scaled = st[bi]["a"]
                for ct in range(CT):
                    ps_at = psA.tile([P, C1], F32, tag="qt")
                    for qi in range(QT):
                        nc.tensor.matmul(
                            ps_at[:, qi * P:(qi + 1) * P],
                            a_scaled[:, qi, ct * P:(ct + 1) * P], ident,
                            start=True, stop=True)
                    nc.vector.tensor_add(
                        out=lhsf[:, ct, :], in0=ps_at, in1=wtf_sb[:, ct, :])
                st[bi]["lhsf"] = lhsf

            def emit_F_group(bi, qi, ng, wide_psum=False):
                lhsf = st[bi]["lhsf"]
                o_sb = osb_pool.tile([P, 4 * NCHUNK], F32, tag="o")
                pcount = 0
                for half in range(2):
                    for sub in range(2 * half, 2 * half + 2):
                        nch = ng * 4 + sub
                        pieces = xb_slices(bi, ct=0, lo=nch * NCHUNK,
                                           width=NCHUNK)
                        # one psum tile + evacuation per contiguous piece
                        # (two parallel accumulation groups must not share
                        # a PSUM bank: start=True zeroes the whole bank)
                        for pj, (off, _) in enumerate(pieces):
                            w = (pieces[pj + 1][0] if pj + 1 < len(pieces)
                                 else NCHUNK) - off
                            # after the BC loops retire, their PSUM banks
                            # are free — rotate through 4 banks in the tail
                            if wide_psum and pcount % 2 == 1:
                                ps_o_w = psA.tile([P, C], F32, tag="xt",
                                                  name="ps_o_w")
                                ps_o = ps_o_w[:, :NCHUNK]
                            else:
                                ps_o = psC.tile([P, NCHUNK], F32, tag="po",
                                                name="ps_o")
                            pcount += 1
                            for ct in range(CT):
                                rhs = xb_slices(
                                    bi, ct, nch * NCHUNK + off, w)[0][1]
                                nc.tensor.matmul(
                                    ps_o[:, :w],
                                    lhsf[:, ct, qi * P:(qi + 1) * P],
                                    rhs,
                                    start=(ct == 0), stop=(ct == CT - 1))
                            oslice = o_sb[:, sub * NCHUNK + off:
                                          sub * NCHUNK + off + w]
                            if (sub + pj) % 2 == 0:
                                nc.scalar.add(out=oslice, in_=ps_o[:, :w],
                                              add=bq_sb[:, qi:qi + 1])
                            else:
                                nc.vector.tensor_scalar_add(
                                    oslice, ps_o[:, :w],
                                    bq_sb[:, qi:qi + 1])
                    nc.sync.dma_start(
                        out=out_r[bi, :, qi,
                                  (ng * 4 + 2 * half) * NCHUNK:
                                  (ng * 4 + 2 * half + 2) * NCHUNK],
                        in_=o_sb[:, 2 * half * NCHUNK:
                                 (2 * half + 2) * NCHUNK])

            # ---- HAM warm-up: ~3.5us of dummy matmuls on the identity while
            # the first x chunk is still in flight, so real matmuls start at
            # 2.4 GHz instead of paying the cold-clock ramp.
            # single accumulating tile => no inter-warmup semaphores; PE
            # streams these back-to-back and trips the HAM busy window.
            ps_w = psC.tile([P, NCHUNK], F32, tag="po", name="warm")
            NWARM = 48
            for wj in range(NWARM):
                nc.tensor.matmul(ps_w[:, :P], ident, ident,
                                 start=(wj == 0), stop=(wj == NWARM - 1))

            # ---- the schedule ----
            issue_x_dma(0)
            issue_x_dma(1)
            emit_BC_range(0, 0, NT)
            emit_softmax(0)
            emit_BC_range(1, 0, NT // 2)
            emit_ATcombine(0)
            emit_F_group(0, 0, 0)
            emit_F_group(0, 0, 1)
            emit_BC_range(1, NT // 2, NT)
            emit_softmax(1)
            emit_F_group(0, 1, 0)
            emit_ATcombine(1)
            emit_F_group(0, 1, 1)
            for qi in range(QT):
                for ng in range(2):
                    emit_F_group(1, qi, ng, wide_psum=True)
    nc.compile()
    return nc


_NC_CACHE = None


def _get_nc():
    global _NC_CACHE
    if _NC_CACHE is None:
        _NC_CACHE = build_nc()
    return _NC_CACHE


def make_in_maps(x, conv_w, conv_b, gamma):
    B = x.shape[0]
    xs_full = np.ascontiguousarray(x.reshape(B, C, HW), dtype=np.float32)
    wm = conv_w.reshape(C1, C).astype(np.float32)
    wt = np.ascontiguousarray(wm.T)                    # [C, C1]
    wt_tiled = np.ascontiguousarray(
        wt.reshape(CT, P, C1).transpose(1, 0, 2))      # [P, CT, C1]
    wtb_tiled = wt_tiled.astype(ml_dtypes.bfloat16)
    b_np = conv_b.astype(np.float32)
    bbc = np.ascontiguousarray(np.broadcast_to(b_np[None, :], (P, C1)))
    bq = np.ascontiguousarray(b_np.reshape(QT, P).T)   # [P, QT]
    gam = np.ascontiguousarray(
        np.broadcast_to(gamma.astype(np.float32).reshape(1, 1), (P, 1)))
    in_maps = []
    for ci in range(N_CORES):
        in_maps.append({
            "xs": np.ascontiguousarray(xs_full[NB * ci:NB * (ci + 1)]),
            "wt_f": wt_tiled,
            "wt_b": wtb_tiled,
            "bbc": bbc,
            "bq": bq,
            "gam": gam,
        })
    return in_maps


def kernel(x, conv_w, conv_b, gamma, trace=False):
    """Full inputs in, full output out. Shards batch over 8 NeuronCores."""
    nc = _get_nc()
    in_maps = make_in_maps(x, conv_w, conv_b, gamma)
    res = run_bass_kernel_spmd(nc, in_maps, core_ids=list(range(N_CORES)),
                               trace=trace)
    outs = [r["out"].reshape(NB, C1, 64, 64) for r in res.results]
    full = np.concatenate(outs, axis=0).astype(np.float32)
    if trace:
        kernel.last_results = res
    return full


kernel.last_results = None



# revision 6
# speedup vs baseline: 1.3123x; 1.3123x over previous
"""Trainium2 Bass kernel for CAM-style channel attention module.

Reference computation (per batch b):
    Q  = W @ X + bias          # 1x1 conv: [256,512]@[512,4096] -> [256,4096]
    E  = Q @ X^T / sqrt(4096)  # [256,512] channel-attention energy
    A  = softmax(E, axis=-1)
    out = gamma * (A @ X) + Q  # residual

Two algebraic tricks:
 1. Residual fusion:  gamma*(A@X) + (W@X + b) = (W + gamma*A) @ X + b
    so the final stage is one fused matmul with combined weights.
 2. Gram factorization of the energy:
        E*64 = Q @ X^T = W @ (X X^T) + b (1^T X^T) = W @ G + b s^T
    G = X X^T is a 512x512 SYMMETRIC matrix: compute only the lower
    triangle on the PE (10/16 of the full cost), mirror the off-diag
    blocks with 6 cheap PE transposes, and recover E with a tiny
    [256,512]@[512,512] matmul.  The row-sum vector s falls out of the
    same matmuls via a ones-column prepended to the host-provided X^T.

Everything runs in fp16 (fp32 PSUM accumulation): fp16 has 8x finer
mantissa than bf16, cutting numeric error ~4x vs a bf16 design, and
the output is written fp16 (half the store traffic).  fp8 was
evaluated and rejected: the softmax here is highly peaked, so logit
noise from e4m3 quantization (~0.07 abs) blows the 2e-2 budget.

Device strategy: 8 NeuronCores, data-parallel over batch, 2 per core.
Host pre-transposes/casts X (both [c,n] fp16 for the final stage and
ones-augmented [n,c] fp16 for the Gram stage) so the device does zero
layout work.  Per-core PE time ~2x34us vs ~2x48us for the direct
Q/E/transpose formulation.

G row passes (lower triangle, xta col 0 = ones -> s segments):
    p0: row3 cols 0:384+s3   p1: row2 0:384+s2   p2: row1 0:256+s1
    p3: row0 0:128+s0        p4: row3 diag 384:512 (no ones)
p4 is last so the s-column is complete one full pass before the E
stage needs s as a [1,512] row (PE transpose + 4 tiny SBUF-to-SBUF
DMA gathers have ~3us of latency to hide).  Mirrors/E-contraction
matmuls are hooked into the middle of later passes' instruction
streams so the PE never waits on an evacuation it just triggered.

PSUM budget (8 banks): psG tag "g" bufs=5 (G passes + E accumulators;
the 7-allocations-per-batch rotation is timed so every bank reuse
lands after the prior group's evacuation), psF tag "o" bufs=3
(warmup / mirrors / A^T / final-stage chunks).
"""

import numpy as np

import concourse.bass as bass
import concourse.tile as tile
from concourse import bacc, mybir
from concourse.bass_utils import run_bass_kernel_spmd

P = 128
NB = 2        # batches per core (B=16 over 8 cores)
C = 512       # input channels
C1 = 256      # conv output channels
HW = 4096     # H*W
CT = C // P   # 4 c-tiles
NT = HW // P  # 32 n-tiles
QT = C1 // P  # 2 q-tiles
XTW = 513     # xta row: [ones | X^T row]
F32 = mybir.dt.float32
F16 = mybir.dt.float16
ESCALE = 1.0 / 64.0  # 1/sqrt(HW)

N_CORES = 8

# (ci, rhs_lo, rhs_hi) over xta columns; xta col 1+c is X^T col c.
G_PASSES = [
    (3, 0, 385),    # s[384:512] + G[3-block, 0:384]
    (2, 0, 385),    # s[256:384] + G[2-block, 0:384]
    (1, 0, 257),    # s[128:256] + G[1-block, 0:256]
    (0, 0, 129),    # s[0:128]   + G[0-block, 0:128]
    (3, 385, 513),  # G[3-block, 384:512] (diag block, no ones col)
]
# hooked items fire mid-tail of pass <key> (0-based), after the
# previous pass's evacuation has had time to complete.
# mirror (dst, src): gsb[:, dst, src-block] <- T(gsb[:, src, dst-block])
G_MIRRORS = {1: [(0, 3), (1, 3), (2, 3)],
             2: [(0, 2), (1, 2)],
             3: [(0, 1)]}
G_ECT = {2: 2, 3: 1, 4: 0}  # pass idx -> E ct emitted mid that pass
# E ct 3 needs p4 (last pass) and is emitted right after emit_G.


def build_nc():
    nc = bacc.Bacc("TRN2", target_bir_lowering=False, debug=False,
                   num_devices=N_CORES)

    xta_d = nc.dram_tensor("xta", [NB, P, NT, XTW], F16,
                           kind="ExternalInput").ap()
    x16_d = nc.dram_tensor("x16", [NB, P, CT, HW], F16,
                           kind="ExternalInput").ap()
    wt16_d = nc.dram_tensor("wt16", [P, CT, C1], F16,
                            kind="ExternalInput").ap()
    brow_d = nc.dram_tensor("brow", [1, C1], F16, kind="ExternalInput").ap()
    bq_d = nc.dram_tensor("bq", [P, QT], F32, kind="ExternalInput").ap()
    gam_d = nc.dram_tensor("gam", [P, 1], F32, kind="ExternalInput").ap()
    out_d = nc.dram_tensor("out", [NB, C1, HW], F16,
                           kind="ExternalOutput").ap()

    ident_dram = nc.inline_tensor(np.eye(P, dtype=np.float16), name="ident")

    with tile.TileContext(nc) as tc:
        with (
            tc.tile_pool(name="const", bufs=1) as const,
            tc.tile_pool(name="xta_p", bufs=NB) as xta_pool,
            tc.tile_pool(name="x16_p", bufs=NB) as x16_pool,
            tc.tile_pool(name="gsb_p", bufs=NB) as gsb_pool,
            tc.tile_pool(name="sm", bufs=NB) as sm_pool,
            tc.tile_pool(name="svec", bufs=NB) as svec_pool,
            tc.tile_pool(name="lhsf_p", bufs=NB) as lhsf_pool,
            tc.tile_pool(name="osb_p", bufs=3) as osb_pool,
            tc.tile_pool(name="psG", bufs=5, space="PSUM") as psG,
            tc.tile_pool(name="psF", bufs=3, space="PSUM") as psF,
        ):
            # ---- constants ----
            ident = const.tile([P, P], F16)
            nc.sync.dma_start(out=ident, in_=ident_dram.ap())
            wt16 = const.tile([P, CT, C1], F16)
            nc.sync.dma_start(out=wt16, in_=wt16_d)
            brow = const.tile([1, C1], F16)
            nc.sync.dma_start(out=brow, in_=brow_d)
            bq = const.tile([P, QT], F32)
            nc.sync.dma_start(out=bq, in_=bq_d)
            gam = const.tile([P, 1], F32)
            nc.sync.dma_start(out=gam, in_=gam_d)

            out_r = out_d.rearrange("b (t p) n -> b p t n", p=P)

            st = [dict() for _ in range(NB)]

            # ---- input DMAs, all on the sync (SP) queue, in priority
            # order: xta0 (Gram stage batch 0), xta1, then x16 (final
            # stage consumes it much later).
            XCH = 8   # xta chunks (4 nt each)
            for bi in range(NB):
                st[bi]["xta"] = xta_pool.tile([P, NT, XTW], F16, tag="xta",
                                              name=f"xta{bi}")
            for bi in range(NB):
                for k in range(XCH):
                    nt0, nt1 = k * (NT // XCH), (k + 1) * (NT // XCH)
                    nc.sync.dma_start(out=st[bi]["xta"][:, nt0:nt1, :],
                                      in_=xta_d[bi][:, nt0:nt1, :])
            for bi in range(NB):
                st[bi]["x16"] = x16_pool.tile([P, CT, HW], F16, tag="x16",
                                              name=f"x16_{bi}")
            for bi in range(NB):
                for k in range(2):
                    n0, n1 = k * (HW // 2), (k + 1) * (HW // 2)
                    nc.sync.dma_start(out=st[bi]["x16"][:, :, n0:n1],
                                      in_=x16_d[bi][:, :, n0:n1])

            for bi in range(NB):
                st[bi]["gsb"] = gsb_pool.tile([P, CT, C], F16, tag="gsb",
                                              name=f"gsb{bi}")
                st[bi]["scol"] = svec_pool.tile([P, CT], F16, tag="scol",
                                                name=f"scol{bi}")
                st[bi]["srow"] = svec_pool.tile([1, C], F16, tag="srow",
                                                name=f"srow{bi}")

            # ---- HAM warm-up: dummy matmuls while first xta chunks land
            ps_w = psF.tile([P, C], F32, tag="o", name="warm")
            NWARM = 40
            for wj in range(NWARM):
                nc.tensor.matmul(ps_w[:, :P], ident, ident,
                                 start=(wj == 0), stop=(wj == NWARM - 1))

            # ------------- emission helpers ---------------------------
            def ecopy(eng, out, in_):
                if eng is nc.scalar:
                    eng.copy(out=out, in_=in_)
                else:
                    eng.tensor_copy(out=out, in_=in_)

            def g_mm(bi, pi, nt, start, stop):
                ci, lo, hi = G_PASSES[pi]
                xta = st[bi]["xta"]
                nc.tensor.matmul(
                    st[bi]["ps_g"][pi][:, :hi - lo],
                    xta[:, nt, 1 + ci * P:1 + (ci + 1) * P],
                    xta[:, nt, lo:hi],
                    start=start, stop=stop)

            def g_evac(bi, pi, eng):
                ci, lo, hi = G_PASSES[pi]
                ps = st[bi]["ps_g"][pi]
                gsb, scol = st[bi]["gsb"], st[bi]["scol"]
                if lo == 0:
                    ecopy(eng, scol[:, ci:ci + 1], ps[:, 0:1])
                    g0, p0 = 0, 1
                else:
                    g0, p0 = lo - 1, 0
                gw = (hi - lo) - p0
                ecopy(eng, gsb[:, ci, g0:g0 + gw], ps[:, p0:p0 + gw])

            def emit_mirror(bi, dst, src, eng):
                gsb = st[bi]["gsb"]
                ps_m = psF.tile([P, P], F32, tag="o", name=f"ps_m{bi}")
                nc.tensor.matmul(ps_m, gsb[:, src, dst * P:(dst + 1) * P],
                                 ident, start=True, stop=True)
                ecopy(eng, gsb[:, dst, src * P:(src + 1) * P], ps_m)

            def emit_e_ct(bi, ct):
                # E psum group: opened at the first ct, closed by emit_bs
                if "ps_e" not in st[bi]:
                    st[bi]["ps_e"] = [
                        psG.tile([P, C], F32, tag="g", name=f"ps_e{bi}{qi}")
                        for qi in range(QT)]
                    st[bi]["e_started"] = False
                first = not st[bi]["e_started"]
                st[bi]["e_started"] = True
                for qi in range(QT):
                    nc.tensor.matmul(
                        st[bi]["ps_e"][qi],
                        wt16[:, ct, qi * P:(qi + 1) * P],
                        st[bi]["gsb"][:, ct, :],
                        start=first, stop=False)

            def emit_s_chain(bi):
                # scol [128,4] --PE transpose--> [4,128] --evac-->
                # --4 tiny SBUF DMAs (vector queue)--> srow [1,512]
                ps_t = psF.tile([4, P], F32, tag="o", name=f"ps_t{bi}")
                nc.tensor.matmul(ps_t, st[bi]["scol"], ident,
                                 start=True, stop=True)
                stt = svec_pool.tile([4, P], F16, tag="st", name=f"st{bi}")
                nc.scalar.copy(out=stt, in_=ps_t)
                nc.gpsimd.dma_start(out=st[bi]["srow"][0:1, :], in_=stt)

            def emit_bs(bi):
                # rank-1 b s^T accumulated into the E psum; closes group
                for qi in range(QT):
                    nc.tensor.matmul(
                        st[bi]["ps_e"][qi],
                        brow[0:1, qi * P:(qi + 1) * P],
                        st[bi]["srow"][0:1, :],
                        start=False, stop=True)

            def emit_G(bi, split, extra_hooks=None):
                # nt-outer prefix (needs 5 psum banks, DMA-streamable),
                # then per-pass tails with hooked mirror/E interleaves.
                st[bi]["ps_g"] = [
                    psG.tile([P, C], F32, tag="g", name=f"ps_g{bi}{pi}")
                    for pi in range(len(G_PASSES))]
                for nt in range(split):
                    for pi in range(len(G_PASSES)):
                        g_mm(bi, pi, nt, start=(nt == 0), stop=False)
                evac_rr = [nc.vector, nc.scalar]
                for pi in range(len(G_PASSES)):
                    tail = list(range(split, NT))
                    for k, nt in enumerate(tail):
                        g_mm(bi, pi, nt, start=(split == 0 and nt == 0),
                             stop=(nt == NT - 1))
                        if k == min(4, len(tail) // 2):
                            for dst, src in G_MIRRORS.get(pi, []):
                                eng = nc.vector if (dst + src) % 2 else \
                                    nc.scalar
                                emit_mirror(bi, dst, src, eng)
                            if pi in G_ECT:
                                emit_e_ct(bi, G_ECT[pi])
                            if pi == len(G_PASSES) - 1:
                                emit_s_chain(bi)
                            if extra_hooks and pi in extra_hooks:
                                for fn in extra_hooks[pi]:
                                    fn()
                    g_evac(bi, pi, evac_rr[pi % 2])
                emit_e_ct(bi, 3)

            def emit_softmax(bi):
                a_scaled = sm_pool.tile([P, QT, C], F16, tag="a",
                                        name=f"a_scaled{bi}")
                for qi in range(QT):
                    ps_e = st[bi]["ps_e"][qi]
                    mx = sm_pool.tile([P, 1], F32, tag="mx")
                    nc.vector.reduce_max(mx, ps_e,
                                         axis=mybir.AxisListType.X,
                                         negate=True)
                    nbias = sm_pool.tile([P, 1], F32, tag="nb")
                    nc.vector.tensor_scalar_mul(nbias, mx, ESCALE)
                    a_f = sm_pool.tile([P, C], F32, tag="af")
                    rs = sm_pool.tile([P, 1], F32, tag="rs")
                    nc.scalar.activation(
                        out=a_f, in_=ps_e,
                        func=mybir.ActivationFunctionType.Exp,
                        bias=nbias, scale=ESCALE, accum_out=rs)
                    rc = sm_pool.tile([P, 1], F32, tag="rc")
                    nc.vector.reciprocal(rc, rs)
                    sc = sm_pool.tile([P, 1], F32, tag="sc")
                    nc.vector.tensor_mul(sc, rc, gam)
                    nc.vector.tensor_scalar_mul(a_scaled[:, qi, :], a_f, sc)
                st[bi]["a"] = a_scaled

            def emit_AT(bi):
                lhsf = lhsf_pool.tile([P, CT, C1], F16, name=f"lhsf{bi}")
                a_scaled = st[bi]["a"]
                for ct in range(CT):
                    ps_at = psF.tile([P, C1], F32, tag="o", name="ps_at")
                    for qi in range(QT):
                        nc.tensor.matmul(
                            ps_at[:, qi * P:(qi + 1) * P],
                            a_scaled[:, qi, ct * P:(ct + 1) * P], ident,
                            start=True, stop=True)
                    nc.vector.tensor_add(
                        out=lhsf[:, ct, :], in0=ps_at, in1=wt16[:, ct, :])
                st[bi]["lhsf"] = lhsf

            def emit_F(bi, qi, out_q, hooks=None):
                # final = lhsf^T @ X (+b), 8 chunks of 512 per q-tile
                lhsf = st[bi]["lhsf"]
                x16 = st[bi]["x16"]
                for pair in range(4):
                    o_sb = osb_pool.tile([P, 2 * C], F16, tag="osb")
                    for half in range(2):
                        nch = pair * 2 + half
                        ps_o = psF.tile([P, C], F32, tag="o", name="ps_o")
                        for ct in range(CT):
                            nc.tensor.matmul(
                                ps_o, lhsf[:, ct, qi * P:(qi + 1) * P],
                                x16[:, ct, nch * C:(nch + 1) * C],
                                start=(ct == 0), stop=(ct == CT - 1))
                        if hooks and (pair, half) in hooks:
                            for fn in hooks[(pair, half)]:
                                fn()
                        osl = o_sb[:, half * C:(half + 1) * C]
                        if half == 0:
                            nc.scalar.add(out=osl, in_=ps_o,
                                          add=bq[:, qi:qi + 1])
                        else:
                            nc.vector.tensor_scalar_add(
                                osl, ps_o, bq[:, qi:qi + 1])
                    out_q.dma_start(
                        out=out_r[bi, :, qi, pair * 2 * C:(pair + 1) * 2 * C],
                        in_=o_sb)

            # ------------------- the schedule -------------------------
            # batch 0: nt-outer prefix of 24 (xta0 still streaming in),
            # short pass tails carry the mirror/E interleaves.
            emit_G(0, 24)
            # batch 1: fully pass-outer (xta1 resident by then; 2-3 bank
            # rotation).  bs0+softmax0 hook mid pass-1 of G1 so the s0
            # DMA-gather latency hides under PE work.
            def bs_sm0():
                emit_bs(0)
                emit_softmax(0)
            emit_G(1, 0, extra_hooks={1: [bs_sm0]})
            emit_AT(0)

            def bs_sm1():
                emit_bs(1)
                emit_softmax(1)
            emit_F(0, 0, nc.scalar, hooks={(3, 1): [bs_sm1]})
            emit_F(0, 1, nc.scalar)
            emit_AT(1)
            emit_F(1, 0, nc.sync)
            emit_F(1, 1, nc.sync)
    nc.compile()
    return nc


_NC_CACHE = None


def _get_nc():
    global _NC_CACHE
    if _NC_CACHE is None:
        _NC_CACHE = build_nc()
    return _NC_CACHE


def make_in_maps(x, conv_w, conv_b, gamma):
    B = x.shape[0]
    xs = np.ascontiguousarray(x.reshape(B, C, HW), dtype=np.float32)
    Wm = conv_w.reshape(C1, C).astype(np.float32)
    wt16 = np.ascontiguousarray(
        Wm.T.reshape(CT, P, C1).transpose(1, 0, 2)).astype(np.float16)
    b_np = conv_b.astype(np.float32)
    brow = b_np.reshape(1, C1).astype(np.float16)
    bq = np.ascontiguousarray(b_np.reshape(QT, P).T).astype(np.float32)
    gam = np.ascontiguousarray(
        np.broadcast_to(gamma.astype(np.float32).reshape(1, 1), (P, 1)))

    in_maps = []
    for ci in range(N_CORES):
        xta = np.empty((NB, P, NT, XTW), dtype=np.float16)
        x16 = np.empty((NB, P, CT, HW), dtype=np.float16)
        for bi in range(NB):
            Xb = xs[NB * ci + bi]                       # [C, HW] f32
            xta[bi, :, :, 0] = 1.0
            # xta[p, nt, 1+c] = X[c, nt*128+p]
            xta[bi, :, :, 1:] = Xb.reshape(C, NT, P).transpose(2, 1, 0)
            # x16[p, ct, n] = X[ct*128+p, n]
            x16[bi] = Xb.reshape(CT, P, HW).transpose(1, 0, 2)
        in_maps.append({
            "xta": np.ascontiguousarray(xta),
            "x16": np.ascontiguousarray(x16),
            "wt16": wt16,
            "brow": brow,
            "bq": bq,
            "gam": gam,
        })
    return in_maps


def kernel(x, conv_w, conv_b, gamma, trace=False):
    """Full inputs in, full output out. Shards batch over 8 NeuronCores."""
    nc = _get_nc()
    in_maps = make_in_maps(x, conv_w, conv_b, gamma)
    res = run_bass_kernel_spmd(nc, in_maps, core_ids=list(range(N_CORES)),
                               trace=trace)
    outs = [np.asarray(r["out"]).astype(np.float32).reshape(NB, C1, 64, 64)
            for r in res.results]
    full = np.concatenate(outs, axis=0)
    if trace:
        kernel.last_results = res
    return full


kernel.last_results = None
